# revision 18
# baseline (speedup 1.0000x reference)
"""TRN2 Bass kernel for nn_MultiHeadAttention_63977832841803 (sparse_attention).

Reference computation (H=8 heads, B=16, NQ=G=512, D=512, KD=VD=64, E=512):
  4 branches c: Q_c/K_c/V_c = per-head projections of q; s_c = (1/8) Q_c K_c^T;
  softmax over the concatenated (4*G) axis with per-branch masks (shared across
  heads); heads = sum_c attn_c V_c; out = sum_h heads_h @ W_out[h].

Sharding: pure data-parallel over batch B across 8 cores (2 batches/core), all
heads local, no collectives.

Per-core layout (everything "transposed": tokens on the free axis):
  qT[d, n], Q^T/K^T[h*64+k, n], V[g, h*65+v] (65th col = ones so the attention
  denominator falls out of the same PSUM accumulation), scores s^T[g, n] on PE
  (head pairs run concurrently in row-groups 0:64 / 64:128), exp on ScalarE
  (scale=1/8 folded in; no max-subtraction needed since |s| < ~25), mask
  multiply on VectorE, AV + denominator on PE into [65, n] PSUM, normalization
  via reciprocal + K=1 broadcast matmuls, final W_out contraction on PE with
  heads^T stacked [h*64+v, n].

Host-side preprocessing (cheap numpy): batch shard, transpose q and the masks,
pack weights per-branch as [D, H*64], cast to bf16.
"""

import numpy as np
import ml_dtypes

import concourse.bass as bass  # noqa: F401  (engine types referenced via nc)
import concourse.mybir as mybir
from concourse import bacc, tile
from concourse.bass_utils import run_bass_kernel_spmd

BF16 = mybir.dt.bfloat16
F32 = mybir.dt.float32
F32R = mybir.dt.float32r
AF = mybir.ActivationFunctionType

H, B, G, NQ = 8, 16, 512, 512
D, KD, VD, E = 512, 64, 64, 512
NORM = 1.0 / float(np.sqrt(KD))
NCORES = 8
BLOC = B // NCORES  # batches per core
NB = ml_dtypes.bfloat16

DC, GC, HP, NCH = 4, 4, 4, 4  # d-chunks, g-chunks, head-pairs, n-chunks


def build_kernel(reps=1):
    nc = bacc.Bacc()

    qt = nc.dram_tensor("qt", [BLOC, D, NQ], BF16, kind="ExternalInput")
    mt = nc.dram_tensor("mt", [4, BLOC, G, NQ], BF16, kind="ExternalInput")
    wq = nc.dram_tensor("wq", [4, D, H * KD], BF16, kind="ExternalInput")
    wk = nc.dram_tensor("wk", [4, D, H * KD], BF16, kind="ExternalInput")
    wv = nc.dram_tensor("wv", [4, D, H * VD], BF16, kind="ExternalInput")
    wo = nc.dram_tensor("wo", [H * VD, E], BF16, kind="ExternalInput")
    out = nc.dram_tensor("out", [BLOC, NQ, E], F32, kind="ExternalOutput")

    # NOTE: pool `bufs` is per-TAG; totals below are sized so PSUM tags sum to
    # exactly 8 banks (pproj 2 + score 2*2 + av 2) and SBUF stays under the
    # ~192KB/partition tile allocator budget.
    with tile.TileContext(nc) as tc:
        with (
            tc.tile_pool(name="wsb", bufs=1) as wsb,        # persistent weights
            tc.tile_pool(name="qsb", bufs=2) as qsb,        # qT tiles (4 tags)
            tc.tile_pool(name="msb", bufs=1) as msb,        # mask tiles (16 tags)
            tc.tile_pool(name="proj", bufs=1) as projp,     # Q^T + K^T tiles (32 tags)
            tc.tile_pool(name="vaug", bufs=1) as vaugp,     # V tiles (16 tags)
            tc.tile_pool(name="expool", bufs=5) as expool,
            tc.tile_pool(name="small", bufs=1) as small,
            tc.tile_pool(name="headsp", bufs=1) as headsp,  # hu/hn (8 tags)
            tc.tile_pool(name="osb", bufs=2) as osb,
            tc.tile_pool(name="pp", bufs=2, space="PSUM") as pp,    # proj/bcast/wout psum
            tc.tile_pool(name="ps", bufs=2, space="PSUM") as ps,    # score psum [128,1024]
            tc.tile_pool(name="pav", bufs=2, space="PSUM") as pav,  # AV accumulators
        ):
            # ---- persistent constants / weights ----
            # batch-0 qT is the first thing compute needs: DMA it before the weights
            qts_pre = []
            for dc in range(DC):
                t = qsb.tile([128, NQ], BF16, tag=f"qt{dc}", name=f"qtp{dc}")
                nc.sync.dma_start(t[0:64, :], qt[0, 128 * dc : 128 * dc + 64, :])
                nc.sync.dma_start(t[64:128, :], qt[0, 128 * dc + 64 : 128 * (dc + 1), :])
                qts_pre.append(t)
            wq_sb = [[wsb.tile([128, H * KD], BF16, tag=f"wq{c}{dc}", name=f"wq{c}{dc}") for dc in range(DC)] for c in range(4)]
            wk_sb = [[wsb.tile([128, H * KD], BF16, tag=f"wk{c}{dc}", name=f"wk{c}{dc}") for dc in range(DC)] for c in range(4)]
            wv_sb = [[wsb.tile([128, H * VD], BF16, tag=f"wv{c}{dc}", name=f"wv{c}{dc}") for dc in range(DC)] for c in range(4)]
            wo_sb = [wsb.tile([128, E], BF16, tag=f"wo{hc}", name=f"wo{hc}") for hc in range(4)]
            # need-ordered weight DMAs: branch-by-branch so c=0 projections start early
            for c in range(4):
                for dc in range(DC):
                    sl = slice(128 * dc, 128 * (dc + 1))
                    nc.sync.dma_start(wq_sb[c][dc][:], wq[c, sl, :])
                    nc.sync.dma_start(wk_sb[c][dc][:], wk[c, sl, :])
                    nc.sync.dma_start(wv_sb[c][dc][:], wv[c, sl, :])
            for hc in range(4):
                nc.sync.dma_start(wo_sb[hc][:], wo[128 * hc : 128 * (hc + 1), :])
            ones64r = wsb.tile([1, 64], F32, name="ones64r")
            nc.vector.memset(ones64r[:], 1.0)

            for bi, b in enumerate([bb for _ in range(reps) for bb in range(BLOC)]):
                # ---- load qT and masks for this batch ----
                if bi == 0:
                    qts = qts_pre
                else:
                    qts = []
                    for dc in range(DC):
                        t = qsb.tile([128, NQ], BF16, tag=f"qt{dc}")
                        nc.sync.dma_start(t[:], qt[b, 128 * dc : 128 * (dc + 1), :])
                        qts.append(t)
                mts = [[None] * GC for _ in range(4)]
                for c in range(4):
                    for gc in range(GC):
                        t = msb.tile([128, NQ], BF16, tag=f"m{c}{gc}")
                        nc.sync.dma_start(t[:], mt[c, b, 128 * gc : 128 * (gc + 1), :])
                        mts[c][gc] = t

                # ---- projections ----
                # Q^T / K^T: [hk, n] tiles per (branch, head-pair)
                qT = [[None] * HP for _ in range(4)]
                kT = [[None] * HP for _ in range(4)]
                for c in range(4):
                    for hp in range(HP):
                        hsl = slice(128 * hp, 128 * (hp + 1))
                        pq = pp.tile([128, NQ], F32, tag="pproj")
                        for dc in range(DC):
                            nc.tensor.matmul(pq[:], wq_sb[c][dc][:, hsl], qts[dc][:],
                                             start=(dc == 0), stop=(dc == DC - 1))
                        tq = projp.tile([128, NQ], BF16, tag=f"q{c}{hp}")
                        nc.any.tensor_copy(tq[:], pq[:])
                        qT[c][hp] = tq

                        pk = pp.tile([128, NQ], F32, tag="pproj")
                        for dc in range(DC):
                            nc.tensor.matmul(pk[:], wk_sb[c][dc][:, hsl], qts[dc][:],
                                             start=(dc == 0), stop=(dc == DC - 1))
                        tk = projp.tile([128, NQ], BF16, tag=f"k{c}{hp}")
                        nc.any.tensor_copy(tk[:], pk[:])
                        kT[c][hp] = tk

                # V: [g, h*65+v] tiles per (branch, g-chunk), 65th col ones
                vaug = [[None] * GC for _ in range(4)]
                for c in range(4):
                    for gc in range(GC):
                        gsl = slice(128 * gc, 128 * (gc + 1))
                        pv = pp.tile([128, H * VD], F32, tag="pproj")
                        for dc in range(DC):
                            nc.tensor.matmul(pv[:], qts[dc][:, gsl], wv_sb[c][dc][:],
                                             start=(dc == 0), stop=(dc == DC - 1))
                        tv = vaugp.tile([128, H * 65], BF16, tag=f"v{c}{gc}")
                        tv3 = tv[:].rearrange("p (h v) -> p h v", v=65)
                        pv3 = pv[:].rearrange("p (h v) -> p h v", v=64)
                        nc.vector.tensor_copy(tv3[:, :, 0:64], pv3[:, :, :])
                        nc.vector.memset(tv3[:, :, 64], 1.0)
                        vaug[c][gc] = tv

                # ---- attention ----
                den = small.tile([1, H * NQ], F32, tag="den")
                rec = small.tile([1, H * NQ], F32, tag="rec")
                av_sb = [None] * H  # unnormalized heads^T [64, n] bf16 (in [128,512] pair tiles)
                for hp in range(HP):
                    h0, h1 = 2 * hp, 2 * hp + 1
                    pa0 = pav.tile([65, NQ], F32, tag="av")
                    pa1 = pav.tile([65, NQ], F32, tag="av")
                    # AV matmuls are issued one (c,gc) iteration behind their
                    # scores so the first AV of a new head-pair (which waits on
                    # the previous pair's PSUM handoff) never heads the PE FIFO
                    # with no score work queued in front of it.
                    iters = [(c, gc) for c in range(4) for gc in range(GC)]
                    first, last = iters[0], iters[-1]

                    def issue_av(c, gc, exm):
                        st = (c, gc) == first
                        sp = (c, gc) == last
                        nc.tensor.matmul(pa0[:], vaug[c][gc][:, 65 * h0 : 65 * h0 + 65],
                                         exm[:, 0:NQ], start=st, stop=sp)
                        nc.tensor.matmul(pa1[:], vaug[c][gc][:, 65 * h1 : 65 * h1 + 65],
                                         exm[:, NQ : 2 * NQ], start=st, stop=sp)

                    pending = None  # (c, gc, exm) awaiting AV issue
                    for c, gc in iters:
                        gsl = slice(128 * gc, 128 * (gc + 1))
                        sc = ps.tile([128, 2 * NQ], F32, tag="score")
                        nc.tensor.matmul(sc[:, 0:NQ], kT[c][hp][0:64, gsl],
                                         qT[c][hp][0:64, :], start=True, stop=True)
                        nc.tensor.matmul(sc[:, NQ : 2 * NQ], kT[c][hp][64:128, gsl],
                                         qT[c][hp][64:128, :], start=True, stop=True)
                        ex = expool.tile([128, 2 * NQ], BF16, tag="ex")
                        nc.scalar.activation(ex[:], sc[:], AF.Exp, scale=NORM)
                        exm = ex
                        nc.vector.tensor_mul(exm[:, 0:NQ], ex[:, 0:NQ], mts[c][gc][:])
                        nc.vector.tensor_mul(exm[:, NQ : 2 * NQ], ex[:, NQ : 2 * NQ], mts[c][gc][:])
                        if pending is not None:
                            issue_av(*pending)
                        pending = (c, gc, exm)
                    issue_av(*pending)
                    # stash denominators + unnormalized heads, release PSUM
                    nc.vector.tensor_copy(den[0:1, NQ * h0 : NQ * (h0 + 1)], pa0[64:65, :])
                    nc.vector.tensor_copy(den[0:1, NQ * h1 : NQ * (h1 + 1)], pa1[64:65, :])
                    # per-hp reciprocal so the tail only waits on hp=3's
                    nc.vector.reciprocal(rec[0:1, NQ * h0 : NQ * (h1 + 1)],
                                         den[0:1, NQ * h0 : NQ * (h1 + 1)])
                    hu = headsp.tile([128, NQ], BF16, tag=f"hu{hp}")
                    nc.scalar.copy(hu[0:64, :], pa0[0:64, :])
                    nc.vector.tensor_copy(hu[64:128, :], pa1[0:64, :])
                    av_sb[h0] = hu
                    av_sb[h1] = hu

                # ---- normalize: heads^T * (1/den) broadcast along partitions ----
                headsN = [None] * HP
                for hp in range(HP):
                    h0, h1 = 2 * hp, 2 * hp + 1
                    pb = pp.tile([128, NQ], F32, tag="pproj")
                    nc.tensor.matmul(pb[0:64, :], ones64r[:], rec[0:1, NQ * h0 : NQ * (h0 + 1)],
                                     start=True, stop=True)
                    nc.tensor.matmul(pb[64:128, :], ones64r[:], rec[0:1, NQ * h1 : NQ * (h1 + 1)],
                                     start=True, stop=True)
                    hn = headsp.tile([128, NQ], BF16, tag=f"hn{hp}")
                    nc.vector.tensor_mul(hn[:], av_sb[2 * hp][:], pb[:])
                    headsN[hp] = hn

                # ---- final W_out contraction: out[n, e] ----
                for nch in range(NCH):
                    nsl = slice(128 * nch, 128 * (nch + 1))
                    pw = pp.tile([128, E], F32, tag="pproj")
                    for hc in range(4):
                        nc.tensor.matmul(pw[:], headsN[hc][:, nsl], wo_sb[hc][:],
                                         start=(hc == 0), stop=(hc == 3))
                    ot = osb.tile([128, E], F32, tag="osb")
                    nc.vector.tensor_copy(ot[:], pw[:])
                    nc.sync.dma_start(out[b, nsl, :], ot[:])

    nc.finalize()
    return nc


_NC_CACHE = None


def _get_nc():
    global _NC_CACHE
    if _NC_CACHE is None:
        _NC_CACHE = build_kernel()
    return _NC_CACHE


def _prep_core_inputs(q, att, grp, spd, wq_np, wk_np, wv_np, wo_np, c0):
    """Host-side shard + transpose + pack for one core's batches [c0, c0+BLOC)."""
    sl = slice(c0, c0 + BLOC)
    qt = np.ascontiguousarray(q[sl].transpose(0, 2, 1)).astype(NB)  # [BLOC, D, NQ]
    # masks in [g, n] orientation per branch: m0=sparse^T, m1=att raw, m2=att^T, m3=group^T
    mt = np.empty((4, BLOC, G, NQ), dtype=NB)
    mt[0] = spd[sl].transpose(0, 2, 1)
    mt[1] = att[sl]
    mt[2] = att[sl].transpose(0, 2, 1)
    mt[3] = grp[sl].transpose(0, 2, 1)
    return {"qt": qt, "mt": mt, "wq": wq_np, "wk": wk_np, "wv": wv_np, "wo": wo_np}


def _pack_w(ws):
    # list of 4 (H, D, Kd) -> [4, D, H*Kd] bf16
    return np.stack([w.transpose(1, 0, 2).reshape(D, -1) for w in ws]).astype(NB)


def kernel(q, att_masks, group_masks, sparse_dist_masks,
           W_query, W_K, W_V, W_Q_ps, W_K_ps, W_V_ps,
           W_Q_sp, W_K_sp, W_V_sp, W_Q_pg, W_K_pg, W_V_pg, W_out,
           _want_results=False):
    q = np.asarray(q, dtype=np.float32)
    att = np.asarray(att_masks).astype(np.float32)
    grp = np.asarray(group_masks).astype(np.float32)
    spd = np.asarray(sparse_dist_masks).astype(np.float32)

    wq_np = _pack_w([np.asarray(w, np.float32) for w in (W_query, W_Q_ps, W_Q_sp, W_Q_pg)])
    wk_np = _pack_w([np.asarray(w, np.float32) for w in (W_K, W_K_ps, W_K_sp, W_K_pg)])
    wv_np = _pack_w([np.asarray(w, np.float32) for w in (W_V, W_V_ps, W_V_sp, W_V_pg)])
    wo_np = np.asarray(W_out, np.float32).reshape(H * VD, E).astype(NB)

    in_maps = [
        _prep_core_inputs(q, att, grp, spd, wq_np, wk_np, wv_np, wo_np, BLOC * i)
        for i in range(NCORES)
    ]
    nc = _get_nc()
    res = run_bass_kernel_spmd(nc, in_maps, list(range(NCORES)))
    out = np.concatenate([res.results[i]["out"] for i in range(NCORES)], axis=0)
    if _want_results:
        return out, res
    return out


# revision 19
# speedup vs baseline: 6.7823x; 6.7823x over previous
"""TRN2 Bass kernel for nn_MultiHeadAttention_63977832841803 (sparse_attention).

Reference computation (H=8 heads, B=16, NQ=G=512, D=512, KD=VD=64, E=512):
  4 branches c: Q_c/K_c/V_c = per-head projections of q; s_c = (1/8) Q_c K_c^T;
  softmax over the concatenated (4*G) axis with per-branch masks (shared across
  heads); heads = sum_c attn_c V_c; out = sum_h heads_h @ W_out[h].

Sharding: pure data-parallel over batch B across 8 cores (2 batches/core), all
heads local, no collectives.

Per-core layout (everything "transposed": tokens on the free axis):
  qT[d, n], Q^T/K^T[h*64+k, n], V[g, h*65+v] (65th col = ones so the attention
  denominator falls out of the same PSUM accumulation), scores s^T[g, n] on PE
  (head pairs run concurrently in row-groups 0:64 / 64:128), exp on ScalarE
  (scale=1/8 folded in; no max-subtraction needed since |s| < ~25), mask
  multiply on VectorE, AV + denominator on PE into [65, n] PSUM, normalization
  via reciprocal + K=1 broadcast matmuls, final W_out contraction on PE with
  heads^T stacked [h*64+v, n].

Host-side preprocessing (cheap numpy): batch shard, transpose q and the masks,
pack weights per-branch as [D, H*64], cast to bf16.
"""

import numpy as np
import ml_dtypes

import concourse.bass as bass  # noqa: F401  (engine types referenced via nc)
import concourse.mybir as mybir
from concourse import bacc, tile
from concourse.bass_utils import run_bass_kernel_spmd

BF16 = mybir.dt.bfloat16
F32 = mybir.dt.float32
F32R = mybir.dt.float32r
AF = mybir.ActivationFunctionType

H, B, G, NQ = 8, 16, 512, 512
D, KD, VD, E = 512, 64, 64, 512
NORM = 1.0 / float(np.sqrt(KD))
NCORES = 8
BLOC = B // NCORES  # batches per core
NB = ml_dtypes.bfloat16

DC, GC, HP, NCH = 4, 4, 4, 4  # d-chunks, g-chunks, head-pairs, n-chunks


def build_kernel(reps=1):
    nc = bacc.Bacc()

    qt = nc.dram_tensor("qt", [BLOC, D, NQ], BF16, kind="ExternalInput")
    mt = nc.dram_tensor("mt", [4, BLOC, G, NQ], BF16, kind="ExternalInput")
    wq = nc.dram_tensor("wq", [4, D, H * KD], BF16, kind="ExternalInput")
    wk = nc.dram_tensor("wk", [4, D, H * KD], BF16, kind="ExternalInput")
    wv = nc.dram_tensor("wv", [4, D, H * VD], BF16, kind="ExternalInput")
    wo = nc.dram_tensor("wo", [H * VD, E], BF16, kind="ExternalInput")
    out = nc.dram_tensor("out", [BLOC, NQ, E], F32, kind="ExternalOutput")

    # NOTE: pool `bufs` is per-TAG; totals below are sized so PSUM tags sum to
    # exactly 8 banks (pproj 2 + score 2*2 + av 2) and SBUF stays under the
    # ~192KB/partition tile allocator budget.
    with tile.TileContext(nc) as tc:
        with (
            tc.tile_pool(name="wsb", bufs=1) as wsb,        # persistent weights
            tc.tile_pool(name="qsb", bufs=2) as qsb,        # qT tiles (4 tags)
            tc.tile_pool(name="msb", bufs=1) as msb,        # mask tiles (16 tags)
            tc.tile_pool(name="proj", bufs=1) as projp,     # Q^T + K^T tiles (32 tags)
            tc.tile_pool(name="vaug", bufs=1) as vaugp,     # V tiles (16 tags)
            tc.tile_pool(name="expool", bufs=5) as expool,
            tc.tile_pool(name="small", bufs=1) as small,
            tc.tile_pool(name="headsp", bufs=1) as headsp,  # hu/hn (8 tags)
            tc.tile_pool(name="osb", bufs=2) as osb,
            tc.tile_pool(name="pp", bufs=2, space="PSUM") as pp,    # proj/bcast/wout psum
            tc.tile_pool(name="ps", bufs=2, space="PSUM") as ps,    # score psum [128,1024]
            tc.tile_pool(name="pav", bufs=2, space="PSUM") as pav,  # AV accumulators
        ):
            # ---- persistent constants / weights ----
            # batch-0 qT is the first thing compute needs: DMA it before the weights
            qts_pre = []
            for dc in range(DC):
                t = qsb.tile([128, NQ], BF16, tag=f"qt{dc}", name=f"qtp{dc}")
                nc.sync.dma_start(t[0:64, :], qt[0, 128 * dc : 128 * dc + 64, :])
                nc.sync.dma_start(t[64:128, :], qt[0, 128 * dc + 64 : 128 * (dc + 1), :])
                qts_pre.append(t)
            wq_sb = [[wsb.tile([128, H * KD], BF16, tag=f"wq{c}{dc}", name=f"wq{c}{dc}") for dc in range(DC)] for c in range(4)]
            wk_sb = [[wsb.tile([128, H * KD], BF16, tag=f"wk{c}{dc}", name=f"wk{c}{dc}") for dc in range(DC)] for c in range(4)]
            wv_sb = [[wsb.tile([128, H * VD], BF16, tag=f"wv{c}{dc}", name=f"wv{c}{dc}") for dc in range(DC)] for c in range(4)]
            wo_sb = [wsb.tile([128, E], BF16, tag=f"wo{hc}", name=f"wo{hc}") for hc in range(4)]
            # need-ordered weight DMAs: branch-by-branch so c=0 projections start early
            for c in range(4):
                for dc in range(DC):
                    sl = slice(128 * dc, 128 * (dc + 1))
                    nc.sync.dma_start(wq_sb[c][dc][:], wq[c, sl, :])
                    nc.sync.dma_start(wk_sb[c][dc][:], wk[c, sl, :])
                    nc.sync.dma_start(wv_sb[c][dc][:], wv[c, sl, :])
            for hc in range(4):
                nc.sync.dma_start(wo_sb[hc][:], wo[128 * hc : 128 * (hc + 1), :])
            ones64r = wsb.tile([1, 64], F32, name="ones64r")
            nc.vector.memset(ones64r[:], 1.0)

            for bi, b in enumerate([bb for _ in range(reps) for bb in range(BLOC)]):
                # ---- load qT and masks for this batch ----
                if bi == 0:
                    qts = qts_pre
                else:
                    qts = []
                    for dc in range(DC):
                        t = qsb.tile([128, NQ], BF16, tag=f"qt{dc}")
                        nc.sync.dma_start(t[:], qt[b, 128 * dc : 128 * (dc + 1), :])
                        qts.append(t)
                mts = [[None] * GC for _ in range(4)]
                for c in range(4):
                    for gc in range(GC):
                        t = msb.tile([128, NQ], BF16, tag=f"m{c}{gc}")
                        nc.sync.dma_start(t[:], mt[c, b, 128 * gc : 128 * (gc + 1), :])
                        mts[c][gc] = t

                # ---- projections ----
                # Q^T / K^T: [hk, n] tiles per (branch, head-pair)
                qT = [[None] * HP for _ in range(4)]
                kT = [[None] * HP for _ in range(4)]
                for c in range(4):
                    for hp in range(HP):
                        hsl = slice(128 * hp, 128 * (hp + 1))
                        pq = pp.tile([128, NQ], F32, tag="pproj")
                        for dc in range(DC):
                            nc.tensor.matmul(pq[:], wq_sb[c][dc][:, hsl], qts[dc][:],
                                             start=(dc == 0), stop=(dc == DC - 1))
                        tq = projp.tile([128, NQ], BF16, tag=f"q{c}{hp}")
                        nc.any.tensor_copy(tq[:], pq[:])
                        qT[c][hp] = tq

                        pk = pp.tile([128, NQ], F32, tag="pproj")
                        for dc in range(DC):
                            nc.tensor.matmul(pk[:], wk_sb[c][dc][:, hsl], qts[dc][:],
                                             start=(dc == 0), stop=(dc == DC - 1))
                        tk = projp.tile([128, NQ], BF16, tag=f"k{c}{hp}")
                        nc.any.tensor_copy(tk[:], pk[:])
                        kT[c][hp] = tk

                # V: [g, h*65+v] tiles per (branch, g-chunk), 65th col ones
                vaug = [[None] * GC for _ in range(4)]
                for c in range(4):
                    for gc in range(GC):
                        gsl = slice(128 * gc, 128 * (gc + 1))
                        pv = pp.tile([128, H * VD], F32, tag="pproj")
                        for dc in range(DC):
                            nc.tensor.matmul(pv[:], qts[dc][:, gsl], wv_sb[c][dc][:],
                                             start=(dc == 0), stop=(dc == DC - 1))
                        tv = vaugp.tile([128, H * 65], BF16, tag=f"v{c}{gc}")
                        tv3 = tv[:].rearrange("p (h v) -> p h v", v=65)
                        pv3 = pv[:].rearrange("p (h v) -> p h v", v=64)
                        nc.vector.tensor_copy(tv3[:, :, 0:64], pv3[:, :, :])
                        nc.vector.memset(tv3[:, :, 64], 1.0)
                        vaug[c][gc] = tv

                # ---- attention ----
                den = small.tile([1, H * NQ], F32, tag="den")
                rec = small.tile([1, H * NQ], F32, tag="rec")
                av_sb = [None] * H  # unnormalized heads^T [64, n] bf16 (in [128,512] pair tiles)
                for hp in range(HP):
                    h0, h1 = 2 * hp, 2 * hp + 1
                    pa0 = pav.tile([65, NQ], F32, tag="av")
                    pa1 = pav.tile([65, NQ], F32, tag="av")
                    first, last = (0, 0), (3, GC - 1)
                    for c in range(4):
                        for gc in range(GC):
                            gsl = slice(128 * gc, 128 * (gc + 1))
                            sc = ps.tile([128, 2 * NQ], F32, tag="score")
                            nc.tensor.matmul(sc[:, 0:NQ], kT[c][hp][0:64, gsl],
                                             qT[c][hp][0:64, :], start=True, stop=True)
                            nc.tensor.matmul(sc[:, NQ : 2 * NQ], kT[c][hp][64:128, gsl],
                                             qT[c][hp][64:128, :], start=True, stop=True)
                            ex = expool.tile([128, 2 * NQ], BF16, tag="ex")
                            nc.scalar.activation(ex[:], sc[:], AF.Exp, scale=NORM)
                            exm = ex
                            nc.vector.tensor_mul(exm[:, 0:NQ], ex[:, 0:NQ], mts[c][gc][:])
                            nc.vector.tensor_mul(exm[:, NQ : 2 * NQ], ex[:, NQ : 2 * NQ], mts[c][gc][:])
                            st = (c, gc) == first
                            sp = (c, gc) == last
                            nc.tensor.matmul(pa0[:], vaug[c][gc][:, 65 * h0 : 65 * h0 + 65],
                                             exm[:, 0:NQ], start=st, stop=sp)
                            nc.tensor.matmul(pa1[:], vaug[c][gc][:, 65 * h1 : 65 * h1 + 65],
                                             exm[:, NQ : 2 * NQ], start=st, stop=sp)
                    # stash denominators + unnormalized heads, release PSUM
                    nc.vector.tensor_copy(den[0:1, NQ * h0 : NQ * (h0 + 1)], pa0[64:65, :])
                    nc.vector.tensor_copy(den[0:1, NQ * h1 : NQ * (h1 + 1)], pa1[64:65, :])
                    # per-hp reciprocal so the tail only waits on hp=3's
                    nc.vector.reciprocal(rec[0:1, NQ * h0 : NQ * (h1 + 1)],
                                         den[0:1, NQ * h0 : NQ * (h1 + 1)])
                    hu = headsp.tile([128, NQ], BF16, tag=f"hu{hp}")
                    nc.scalar.copy(hu[0:64, :], pa0[0:64, :])
                    nc.vector.tensor_copy(hu[64:128, :], pa1[0:64, :])
                    av_sb[h0] = hu
                    av_sb[h1] = hu

                # ---- normalize: heads^T * (1/den) broadcast along partitions ----
                headsN = [None] * HP
                for hp in range(HP):
                    h0, h1 = 2 * hp, 2 * hp + 1
                    pb = pp.tile([128, NQ], F32, tag="pproj")
                    nc.tensor.matmul(pb[0:64, :], ones64r[:], rec[0:1, NQ * h0 : NQ * (h0 + 1)],
                                     start=True, stop=True)
                    nc.tensor.matmul(pb[64:128, :], ones64r[:], rec[0:1, NQ * h1 : NQ * (h1 + 1)],
                                     start=True, stop=True)
                    hn = headsp.tile([128, NQ], BF16, tag=f"hn{hp}")
                    nc.vector.tensor_mul(hn[:], av_sb[2 * hp][:], pb[:])
                    headsN[hp] = hn

                # ---- final W_out contraction: out[n, e] ----
                for nch in range(NCH):
                    nsl = slice(128 * nch, 128 * (nch + 1))
                    pw = pp.tile([128, E], F32, tag="pproj")
                    for hc in range(4):
                        nc.tensor.matmul(pw[:], headsN[hc][:, nsl], wo_sb[hc][:],
                                         start=(hc == 0), stop=(hc == 3))
                    ot = osb.tile([128, E], F32, tag="osb")
                    nc.vector.tensor_copy(ot[:], pw[:])
                    nc.sync.dma_start(out[b, nsl, :], ot[:])

    nc.finalize()
    return nc


_NC_CACHE = None


def _get_nc():
    global _NC_CACHE
    if _NC_CACHE is None:
        _NC_CACHE = build_kernel()
    return _NC_CACHE


def _prep_core_inputs(q, att, grp, spd, wq_np, wk_np, wv_np, wo_np, c0):
    """Host-side shard + transpose + pack for one core's batches [c0, c0+BLOC)."""
    sl = slice(c0, c0 + BLOC)
    qt = np.ascontiguousarray(q[sl].transpose(0, 2, 1)).astype(NB)  # [BLOC, D, NQ]
    # masks in [g, n] orientation per branch: m0=sparse^T, m1=att raw, m2=att^T, m3=group^T
    mt = np.empty((4, BLOC, G, NQ), dtype=NB)
    mt[0] = spd[sl].transpose(0, 2, 1)
    mt[1] = att[sl]
    mt[2] = att[sl].transpose(0, 2, 1)
    mt[3] = grp[sl].transpose(0, 2, 1)
    return {"qt": qt, "mt": mt, "wq": wq_np, "wk": wk_np, "wv": wv_np, "wo": wo_np}


def _pack_w(ws):
    # list of 4 (H, D, Kd) -> [4, D, H*Kd] bf16
    return np.stack([w.transpose(1, 0, 2).reshape(D, -1) for w in ws]).astype(NB)


def kernel(q, att_masks, group_masks, sparse_dist_masks,
           W_query, W_K, W_V, W_Q_ps, W_K_ps, W_V_ps,
           W_Q_sp, W_K_sp, W_V_sp, W_Q_pg, W_K_pg, W_V_pg, W_out,
           _want_results=False):
    q = np.asarray(q, dtype=np.float32)
    att = np.asarray(att_masks).astype(np.float32)
    grp = np.asarray(group_masks).astype(np.float32)
    spd = np.asarray(sparse_dist_masks).astype(np.float32)

    wq_np = _pack_w([np.asarray(w, np.float32) for w in (W_query, W_Q_ps, W_Q_sp, W_Q_pg)])
    wk_np = _pack_w([np.asarray(w, np.float32) for w in (W_K, W_K_ps, W_K_sp, W_K_pg)])
    wv_np = _pack_w([np.asarray(w, np.float32) for w in (W_V, W_V_ps, W_V_sp, W_V_pg)])
    wo_np = np.asarray(W_out, np.float32).reshape(H * VD, E).astype(NB)

    in_maps = [
        _prep_core_inputs(q, att, grp, spd, wq_np, wk_np, wv_np, wo_np, BLOC * i)
        for i in range(NCORES)
    ]
    nc = _get_nc()
    res = run_bass_kernel_spmd(nc, in_maps, list(range(NCORES)))
    out = np.concatenate([res.results[i]["out"] for i in range(NCORES)], axis=0)
    if _want_results:
        return out, res
    return out
